# revision 18
# baseline (speedup 1.0000x reference)
"""Trainium2 Bass kernel for nn_Attention additive-attention problem.

Computation (reference, fp32):
    q = query @ Wq.T + bq                      # [B, H]
    r = ref @ Wr.T + br                        # [B, S, H]
    logits = einsum('bsh,h->bs', tanh(q[:,None,:] + r), V)
    w = softmax(logits, axis=1)                # over S
    out = einsum('bsh,bs->bh', r, w)[:, :, None]

Key identity used: since sum_s w = 1,
    out = (sum_s w_s * ref[s,:]) @ Wr.T + br
so r is only needed inside the tanh; the output reduction runs on ref
directly.

Mapping (per core, batch-parallel over 8 cores, 4 batches each):
  - All layout marshalling happens on the host: ref ships twice, once as a
    natural-layout bf16 copy [s%128, s//128, h] (feeds the weighted ref sum,
    keeping output precision at bf16), and once pre-transposed into the fp8e4
    DoubleRow layout [h%128, h//256, (h//128)%2, s].  Wr ships x64-scaled
    (exact power of two) in the same fp8 DoubleRow layout so its U(+-1/32)
    entries clear the e4m3 subnormal floor; tanh's ACT pass applies
    scale=1/64 to undo it.
  - Main projection r^T = Wr ref^T runs as fp8 DoubleRow matmuls (2 rows per
    cycle, half the PE time of bf16): per (s-super-tile of 1024, oc) two
    accumulating matmuls per 512-col half into a 2-bank PSUM tile.
  - One wide ACT per (super-tile, oc) applies tanh over both banks with the
    per-partition bias qq = q + bq + br and the 1/64 scale fused in.
  - logits^T come from PE matmuls with the tanh tile stationary and V as a
    1-column moving operand (LDWEIGHTS at 4 rows/cycle makes these ~30ns a
    pair), so exp(logits) lands with s on partitions, unnormalized.
  - The weighted ref sum runs as fused scalar_tensor_tensor chains
    ((nat * w) + acc) on the otherwise idle DVE, 4 chunks per chain, then
    one 128->1 ones-matmul per chain accumulates into a batch-long PSUM
    bank.  Chains are emitted one super-tile late so the strict PE FIFO
    never waits on the DVE.
  - Batch epilogues (softmax denominator, normalization, projection through
    WrT + br in bf16) are deferred into the next batch's instruction stream.
"""

import numpy as np
import ml_dtypes
from contextlib import ExitStack

import concourse.bass as bass
import concourse.bacc as bacc
import concourse.tile as tile
from concourse import mybir
from concourse import bass_utils
import concourse.bass_isa as bass_isa
from concourse._compat import with_exitstack

F32 = mybir.dt.float32
BF16 = mybir.dt.bfloat16
FP8 = mybir.dt.float8e4
AF = mybir.ActivationFunctionType
ALU = mybir.AluOpType
PSUM = bass.MemorySpace.PSUM
DR = mybir.MatmulPerfMode.DoubleRow

B, S, H = 32, 4096, 512
NCORES = 8
BPC = B // NCORES          # batches per core = 4
HC = H // 128              # h (and o) chunks = 4
NSC = S // 128             # 128-wide s-chunks per batch = 32
ST2 = 1024                 # s-super-tile width
NST2 = S // ST2            # super-tiles per batch = 4
NCH = ST2 // 128           # s-chunks per super-tile = 8
WR_SCALE = 64.0            # host-side power-of-2 scale on Wr (fp8 range fit)


@with_exitstack
def _body(ctx: ExitStack, tc: tile.TileContext,
          refdr, natc, cb, wq_c, wr_dr, wrt_c, br_f, out):
    nc = tc.nc

    consts = ctx.enter_context(tc.tile_pool(name="consts", bufs=1))
    nat_pool = ctx.enter_context(tc.tile_pool(name="nat", bufs=2))
    ref_pool = ctx.enter_context(tc.tile_pool(name="refdr", bufs=2))
    tanh_pool = ctx.enter_context(tc.tile_pool(name="tanh", bufs=3))
    wb_pool = ctx.enter_context(tc.tile_pool(name="wb", bufs=2))
    small = ctx.enter_context(tc.tile_pool(name="small", bufs=4))
    q_pool = ctx.enter_context(tc.tile_pool(name="q", bufs=10))
    rps = ctx.enter_context(tc.tile_pool(name="rps", bufs=3, space=PSUM))
    lps = ctx.enter_context(tc.tile_pool(name="lps", bufs=1, space=PSUM))
    acc = ctx.enter_context(tc.tile_pool(name="acc", bufs=1, space=PSUM))

    # ---------------- prologue: params on chip (host pre-chunked layouts,
    # all contiguous DMAs).  Order matters on the SP ring: the fp8 weights
    # and the first super-tile of ref come first so the main matmuls start
    # ~2us in; the epilogue-only WrT waits until after batch 0's ref.
    wrdr_sb = consts.tile([128, 2, 2, H], FP8)  # 64*WrT as [h%128, g, i, o]
    nc.gpsimd.dma_start(wrdr_sb[:], wr_dr[:])
    wq_bf = consts.tile([128, HC, H], BF16)
    nc.gpsimd.dma_start(wq_bf[:], wq_c[:])
    # qt + v + bq + br packed into one byte blob (a tiny SWDGE DMA costs ~2us
    # fixed, so five separate ones would stall the q-prologue ~10us)
    cblob = consts.tile([128, 72], mybir.dt.uint8)
    nc.gpsimd.dma_start(cblob[:], cb[:])
    qt_bf = cblob[:, 0:32].bitcast(BF16).rearrange("p (a b) -> p a b", a=HC)
    v_bf = cblob[:, 32:40].bitcast(BF16)        # V as [o%128, oc]
    bq_sb = cblob[:, 40:56].bitcast(F32)
    br_sb = cblob[:, 56:72].bitcast(F32)
    br_row = consts.tile([1, H], F32)
    nc.gpsimd.dma_start(br_row[:], br_f[None, :])

    wrt_bf = consts.tile([128, HC, H], BF16)    # WrT (unscaled) for epilogue;
    # its DMA is issued inside load_batch(0), after the first ref chunk
    qq_sb = consts.tile([128, HC, BPC], F32)    # (q + bq + br)^T as [o%128, oc, b]
    ident = consts.tile([1, 1], F32)
    nc.gpsimd.memset(ident[:], 1.0)
    ones_bf = consts.tile([128, 1], BF16)
    nc.gpsimd.memset(ones_bf[:], 1.0)
    bqbr = consts.tile([128, HC], F32)
    nc.vector.tensor_add(bqbr[:], bq_sb[:], br_sb[:])

    # qq^T[o, b] = sum_h WqT[h, o] * queryT[h, b]  (+ bq + br)
    for oc in range(HC):
        qps = lps.tile([128, BPC], F32, tag="lt")
        for hc in range(HC):
            nc.tensor.matmul(
                qps[:],
                wq_bf[:, hc, oc * 128:(oc + 1) * 128],
                qt_bf[:, hc, :],
                start=(hc == 0),
                stop=(hc == HC - 1),
            )
        nc.vector.tensor_scalar_add(qq_sb[:, oc, :], qps[:], bqbr[:, oc:oc + 1])

    # ---------------- helpers ----------------
    def emit_epilogue(bb, wt_b, t_ps):
        """Softmax denom + projection for batch bb.  Emitted in the middle
        of batch bb+1's instruction stream so the strict PE FIFO never
        stalls waiting for the weight accumulation."""
        dsum = small.tile([128, 1], F32, tag="dsum")
        nc.vector.reduce_sum(dsum[:], wt_b[:], axis=mybir.AxisListType.X)
        dall = small.tile([128, 1], F32, tag="dall")
        nc.gpsimd.partition_all_reduce(dall[:], dsum[:], 128, bass_isa.ReduceOp.add)
        rec = small.tile([128, 1], F32, tag="rec")
        nc.vector.reciprocal(rec[:], dall[:])

        # normalize while evicting (scale = 1/D, fp32) -- on DVE, ACT is busy
        t_sb = small.tile([1, H], F32, tag="t_sb")
        nc.vector.tensor_scalar_mul(t_sb[:], t_ps[:], rec[0:1, 0:1])

        # transpose t to [h, 1] columns for the final projection -- all four
        # into one PSUM tile (no slot recycling between them), one copy out
        tT_bf = small.tile([128, HC], BF16, tag="tT")
        ttp = lps.tile([128, HC], F32, tag="lt")
        for c in range(HC):
            nc.tensor.transpose(ttp[:, c:c + 1], t_sb[0:1, c * 128:(c + 1) * 128], ident[0:1, 0:1])
        nc.vector.tensor_copy(tT_bf[:], ttp[:])

        # out[1, o] = sum_h WrT[h, o] * t[h]  + br
        o_ps = lps.tile([1, H], F32, tag="lt")
        for c in range(HC):
            nc.tensor.matmul(
                o_ps[:],
                tT_bf[:, c:c + 1],
                wrt_bf[:, c, :],
                start=(c == 0),
                stop=(c == HC - 1),
            )
        out_sb = small.tile([1, H], F32, tag="out_sb")
        nc.vector.tensor_tensor(out_sb[:], o_ps[:], br_row[:], op=ALU.add)
        nc.sync.dma_start(out[bb:bb + 1, :], out_sb[:])

    def load_batch(bb, wrt_bf=None):
        # Everything rides the single SP HWDGE ring, in exact consumption
        # order: the SDMA engines round-robin between ACTIVE queues at packet
        # granularity, so a second queue with bigger descriptors would starve
        # this one.  One FIFO ring = full control, ~1 batch of prefetch depth.
        rt = ref_pool.tile([128, NST2, 2, 2, ST2], FP8, tag="refdr", name=f"refdr_{bb}")
        nt = nat_pool.tile([128, NSC, H], BF16, tag="nat", name=f"nat_{bb}")
        if bb == 0:
            # super-tile chunks so the first matmuls start ~2us after the
            # weights; nat quarters so the first chains never wait on the
            # whole 4 MB transfer
            for st2 in range(NST2):
                nc.sync.dma_start(rt[:, st2], refdr[bb, :, st2])
            for quarter in range(4):
                q0 = quarter * (NSC // 4)
                nc.sync.dma_start(nt[:, q0:q0 + NSC // 4, :],
                                  natc[bb, :, q0:q0 + NSC // 4, :])
            if wrt_bf is not None:
                nc.gpsimd.dma_start(wrt_bf[:], wrt_c[:])
        else:
            nc.sync.dma_start(rt[:], refdr[bb])
            nc.sync.dma_start(nt[:, 0:NSC // 2, :], natc[bb, :, 0:NSC // 2, :])
            nc.sync.dma_start(nt[:, NSC // 2:, :], natc[bb, :, NSC // 2:, :])
        return rt, nt

    # ---------------- main loop ----------------
    # Per super-tile, 8 s-chunks of weighted ref sum: 6 go through DVE as
    # two (nat*w)+acc chains of 3, 2 go straight to the PE as w-stationary
    # matmuls.  All 16 per-batch accumulation matmuls (8 ones + 8 direct)
    # land in one batch-long PSUM bank.
    N_ACC = 4 * (NST2 - 1) + NCH
    pending = None
    nxt = load_batch(0, wrt_bf)
    for bb in range(BPC):
        rt, nt = nxt
        if bb + 1 < BPC:
            nxt = load_batch(bb + 1)

        wt_b = wb_pool.tile([128, NSC], F32)    # exp(logits)^T, [s%128, s//128]
        wt8 = wb_pool.tile([128, NSC], BF16, tag="wt8")  # bf16 copy for PE stationary
        t_ps = acc.tile([1, H], F32, tag="acc")  # weighted ref sum (whole batch)
        mm_queue = []                            # deferred accumulation matmuls
        n_done = [0]

        def flush_acc(keep, nt=nt, wt8=wt8, t_ps=t_ps, mm_queue=mm_queue, n_done=n_done):
            while len(mm_queue) > keep:
                kind, arg = mm_queue.pop(0)
                i = n_done[0]
                if kind == 'ones':
                    lhsT, rhs = ones_bf[:], arg[:]
                else:  # direct w-stationary weighted chunk
                    lhsT, rhs = wt8[:, arg:arg + 1], nt[:, arg, :]
                nc.tensor.matmul(
                    t_ps[:], lhsT, rhs,
                    start=(i == 0), stop=(i == N_ACC - 1),
                )
                n_done[0] += 1

        def emit_logits(st2, tanh_t, nt=nt, wt_b=wt_b, wt8=wt8, mm_queue=mm_queue):
            # logits^T[s, 1] per 128-s block: stationary = tanh tile, V moving.
            lt = lps.tile([128, NCH], F32, tag="lt", name=f"lt_{bb}_{st2}")
            for j in range(NCH):
                sj, jj = divmod(j, 4)
                for oc in range(HC):
                    nc.tensor.matmul(
                        lt[:, j:j + 1],
                        tanh_t[:, oc, sj, jj * 128:(jj + 1) * 128],
                        v_bf[:, oc:oc + 1],
                        start=(oc == 0),
                        stop=(oc == HC - 1),
                    )
            nc.scalar.activation(wt_b[:, st2 * NCH:(st2 + 1) * NCH], lt[:], AF.Exp)
            c0 = st2 * NCH if st2 == NST2 - 1 else st2 * NCH + 6
            nc.vector.tensor_copy(wt8[:, c0:(st2 + 1) * NCH], wt_b[:, c0:(st2 + 1) * NCH])

            # weighted ref rows: chunks 6,7 straight to PE; 0-5 as two fused
            # (nat * w) + acc chains on DVE.  For the last super-tile of a
            # batch ALL chunks go to PE -- its w-matmuls wait only on the exp
            # that lands right after the logits drain, so the batch never
            # ends on a DVE chain.
            if st2 == NST2 - 1:
                for k in range(NCH):
                    mm_queue.append(('w', st2 * NCH + k))
                return
            mm_queue.append(('w', st2 * NCH + 6))
            mm_queue.append(('w', st2 * NCH + 7))
            for half in range(2):
                qcur = None
                for k in range(3):
                    sc = st2 * NCH + half * 3 + k
                    qn = q_pool.tile([128, H], BF16, tag="q")
                    if qcur is None:
                        nc.vector.tensor_scalar_mul(qn[:], nt[:, sc, :], wt_b[:, sc:sc + 1])
                    else:
                        nc.vector.scalar_tensor_tensor(
                            qn[:], nt[:, sc, :], wt_b[:, sc:sc + 1], qcur[:],
                            op0=ALU.mult, op1=ALU.add,
                        )
                    qcur = qn
                mm_queue.append(('ones', qcur))

        prev_tanh = None
        for st2 in range(NST2):
            if st2 == 1 and pending is not None:
                emit_epilogue(*pending)
                pending = None
            # main matmul r^T[o, s] in fp8 DoubleRow (+ tanh w/ bias on ACT)
            tanh_t = tanh_pool.tile([128, HC, 2, ST2 // 2], BF16)
            for oc in range(HC):
                ps = rps.tile([128, 2, ST2 // 2], F32)
                for sj in range(2):
                    s0 = sj * 512
                    for g in range(2):
                        nc.tensor.matmul(
                            ps[:, sj, :],
                            wrdr_sb[:, g, :, oc * 128:(oc + 1) * 128],
                            rt[:, st2, g, :, s0:s0 + 512],
                            start=(g == 0),
                            stop=(g == 1),
                            perf_mode=DR,
                        )
                nc.scalar.activation(
                    tanh_t[:, oc], ps[:], AF.Tanh,
                    bias=qq_sb[:, oc, bb:bb + 1], scale=1.0 / WR_SCALE,
                )
            if prev_tanh is not None:
                emit_logits(st2 - 1, prev_tanh)
                flush_acc(8)
            prev_tanh = tanh_t

        emit_logits(NST2 - 1, prev_tanh)
        flush_acc(0)
        pending = (bb, wt_b, t_ps)

    emit_epilogue(*pending)


_NC_CACHE = None


def build_nc():
    global _NC_CACHE
    if _NC_CACHE is not None:
        return _NC_CACHE
    nc = bacc.Bacc("TRN2", target_bir_lowering=False, debug=False, num_devices=NCORES)
    refdr = nc.dram_tensor("refdr", [BPC, 128, NST2, 2, 2, ST2], FP8, kind="ExternalInput").ap()
    natc = nc.dram_tensor("natc", [BPC, 128, NSC, H], BF16, kind="ExternalInput").ap()
    cb = nc.dram_tensor("cb", [128, 72], mybir.dt.uint8, kind="ExternalInput").ap()
    wq_c = nc.dram_tensor("wq_c", [128, HC, H], BF16, kind="ExternalInput").ap()
    wr_dr = nc.dram_tensor("wr_dr", [128, 2, 2, H], FP8, kind="ExternalInput").ap()
    wrt_c = nc.dram_tensor("wrt_c", [128, HC, H], BF16, kind="ExternalInput").ap()
    br_f = nc.dram_tensor("br_f", [H], F32, kind="ExternalInput").ap()
    out = nc.dram_tensor("out", [BPC, H], F32, kind="ExternalOutput").ap()
    with tile.TileContext(nc) as tc:
        _body(tc, refdr, natc, cb, wq_c, wr_dr, wrt_c, br_f, out)
    nc.compile()
    _NC_CACHE = nc
    return nc


def _chunk_po(x):
    """[H(=hc*128+p), N] -> [128, HC, N] (pure layout)."""
    x = np.asarray(x)
    return np.ascontiguousarray(x.reshape(HC, 128, -1).transpose(1, 0, 2))


def _dr_pack(xT):
    """[H(=g*256+i*128+p), N] -> [128, 2, 2, N] fp8 DoubleRow layout."""
    xT = np.asarray(xT, np.float32)
    n = xT.shape[1]
    return np.ascontiguousarray(
        xT.reshape(2, 2, 128, n).transpose(2, 0, 1, 3)
    ).astype(ml_dtypes.float8_e4m3)


def make_in_maps(query, ref, Wq, bq, Wr, br, V):
    """Build per-core input maps (host-side sharding + layout marshalling)."""
    query = np.asarray(query, np.float32)
    ref = np.asarray(ref, np.float32)                      # [B, S, H]
    ref_bf = ref.astype(ml_dtypes.bfloat16)
    natc = np.ascontiguousarray(
        ref_bf.reshape(B, NSC, 128, H).transpose(0, 2, 1, 3)
    )                                                      # [B, 128, NSC, H]
    refdr = np.ascontiguousarray(
        ref.transpose(0, 2, 1)
        .reshape(B, 2, 2, 128, NST2, ST2)
        .transpose(0, 3, 4, 1, 2, 5)
    ).astype(ml_dtypes.float8_e4m3)                        # [B, 128, NST2, 2, 2, ST2]

    wq_c = _chunk_po(np.asarray(Wq, np.float32).T).astype(ml_dtypes.bfloat16)
    wrt_c = _chunk_po(np.asarray(Wr, np.float32).T).astype(ml_dtypes.bfloat16)
    wr_dr = _dr_pack(np.asarray(Wr, np.float32).T * WR_SCALE)
    v_c = np.ascontiguousarray(
        np.asarray(V, np.float32).reshape(HC, 128).T).astype(ml_dtypes.bfloat16)
    bq_c = np.ascontiguousarray(np.asarray(bq, np.float32).reshape(HC, 128).T)
    br_c = np.ascontiguousarray(np.asarray(br, np.float32).reshape(HC, 128).T)
    br_f = np.ascontiguousarray(np.asarray(br, np.float32))
    in_maps = []
    for c in range(NCORES):
        sl = slice(c * BPC, (c + 1) * BPC)
        qtc = _chunk_po(query[sl].T).astype(ml_dtypes.bfloat16)  # [128, HC, BPC]
        cb = np.concatenate([
            np.ascontiguousarray(qtc).view(np.uint8).reshape(128, -1),
            np.ascontiguousarray(v_c).view(np.uint8).reshape(128, -1),
            bq_c.view(np.uint8).reshape(128, -1),
            br_c.view(np.uint8).reshape(128, -1),
        ], axis=1)
        in_maps.append(
            {
                "refdr": np.ascontiguousarray(refdr[sl]),
                "natc": np.ascontiguousarray(natc[sl]),
                "cb": np.ascontiguousarray(cb),
                "wq_c": wq_c,
                "wr_dr": wr_dr,
                "wrt_c": wrt_c,
                "br_f": br_f,
            }
        )
    return in_maps


def run(query, ref, Wq, bq, Wr, br, V, trace=False):
    nc = build_nc()
    in_maps = make_in_maps(query, ref, Wq, bq, Wr, br, V)
    res = bass_utils.run_bass_kernel_spmd(
        nc, in_maps, core_ids=list(range(NCORES)), trace=trace
    )
    outs = [res.results[c]["out"] for c in range(NCORES)]
    full = np.concatenate(outs, axis=0).astype(np.float32)  # [B, H]
    return full[:, :, None], res


def kernel(**inputs):
    out, _ = run(
        inputs["query"], inputs["ref"], inputs["Wq"], inputs["bq"],
        inputs["Wr"], inputs["br"], inputs["V"],
    )
    return out
